# revision 25
# baseline (speedup 1.0000x reference)
"""Llama GQA attention (B=2, S=2048, H=4096, 32 q heads / 8 kv heads, HD=128)
on 8 Trainium2 NeuronCores.

Sharding: DP=2 over batch x TP=4 over heads.
  core c: batch b = c // 4, tp rank r = c % 4
  - owns q heads [8r, 8r+8), kv heads [2r, 2r+2)
  - AllGather (bf16) of attention outputs within each 4-core batch group,
    split in two 4-head pieces triggered mid-attention
  - output projection over the full 4096 attn features for output columns
    [1024r, 1024(r+1)) -> disjoint outputs, host concatenates.

All inputs are pre-cast/pre-transposed to the on-chip layout on the HOST
(bf16 weights/X^T), so the device does zero dtype conversion and reads
every operand exactly once per use:
  xt:  [128, kt*S + t]   X^T with hidden dim split into 32 k-tiles
  w:   [128, panel*4096] 20 weight panels (8 wq, 2 wk, 2 wv^T, 8 wo),
                          each [128 k-part, kt*128 + c]
On-chip layout is fully "transposed" ([feature, token]); V^T is computed
with free=512 matmuls like K^T and flipped to V via PE transposes.
Causal masking: per q-chunk c only k-tiles 0..4c+3 are touched; the 4
diagonal tiles compute only the surviving q-range (free = 512-128j) plus
one static [128,128] triangular mask on the leading square (GpSimd).
Softmax skips max-subtraction (scores are O(8), exp fits bf16), the
denominator accumulates on the PE via a ones-matmul, and the reciprocal
uses the fast DVE approximation (18 bits).

Schedule: all four projections first (weight panels stream from DRAM
with a 3-deep prefetch, X^T chunks load between them), then attentions
ordered so the cheapest (chunk 0) runs last, with out-projections
interleaved late: attn1, attn2, attn3, op1, attn0, op2, op3, op0.
Each AllGather piece (triggered after heads 3 and 7 of its attention)
gets 130-170us of covering work, making the kernel robust to
collective-latency variance. The af loads carry tile_wait_until floors
so the list scheduler (whose sim has no collective-latency model) does
not hoist af-consuming matmuls ahead of available attention work.
"""

import sys

for _p in ("/opt/trn_rl_repo",):
    if _p not in sys.path:
        sys.path.append(_p)

import numpy as np
import ml_dtypes

import concourse.bacc as bacc
import concourse.mybir as mybir
import concourse.tile as tile
from concourse.bass_utils import run_bass_kernel_spmd

F32 = mybir.dt.float32
BF16 = mybir.dt.bfloat16

B, S, H = 2, 2048, 4096
NH, NKV, HD = 32, 8, 128
N_CORES = 8
TP = 4
GROUPS = [[0, 1, 2, 3], [4, 5, 6, 7]]

HL = NH // TP          # 8 local q heads
KVL = NKV // TP        # 2 local kv heads
QCOLS = HL * HD        # 1024 local q cols
OC = H // TP           # 1024 local out cols

TC = 512               # token chunk (= one attention q-block)
NCHUNK = S // TC       # 4
KT = H // 128          # 32 contraction tiles
SCALE = float(HD ** -0.5)
NPANEL = 20            # 8 wq + 2 wk + 2 wv + 8 wo
PANW = KT * 128        # 4096 free cols per weight panel

LAST_RESULT = None
_BUILT = {}


def _build():
    nc = bacc.Bacc("TRN2", debug=False, num_devices=N_CORES)

    xt_d = nc.dram_tensor("xt", [128, KT * S], BF16, kind="ExternalInput").ap()
    w_d = nc.dram_tensor("w_all", [128, NPANEL * PANW], BF16,
                         kind="ExternalInput").ap()
    cos_d = nc.dram_tensor("cos_t", [HD, S], F32, kind="ExternalInput").ap()
    sin_d = nc.dram_tensor("sin_t", [HD, S], F32, kind="ExternalInput").ap()
    mask_d = nc.dram_tensor("maskb", [128, 128], BF16, kind="ExternalInput").ap()
    ones_d = nc.dram_tensor("onesb", [128, 128], BF16, kind="ExternalInput").ap()
    eye_d = nc.dram_tensor("eyeb", [128, 128], BF16, kind="ExternalInput").ap()
    out_d = nc.dram_tensor("out_t", [OC, S], BF16, kind="ExternalOutput").ap()

    with tile.TileContext(nc) as tc:
        with tc.tile_pool(name="sb", bufs=1) as sb, \
             tc.tile_pool(name="ps", bufs=1, space="PSUM") as ps, \
             tc.tile_pool(name="dr", bufs=1, space="DRAM") as dr:

            # ---- persistent tiles ----
            cos_sb = sb.tile([HD, S], F32)
            sin_sb = sb.tile([HD, S], F32)
            mask_sb = sb.tile([128, 128], BF16)
            ones_sb = sb.tile([128, 128], BF16)
            eye_sb = sb.tile([128, 128], BF16)
            ktb = sb.tile([128, KVL * S], BF16)            # roped K^T [d, kv*S+t]
            vb = sb.tile([128, (S // 128) * KVL * 128], BF16)  # V [t, tt*256+d]
            nc.sync.dma_start(cos_sb[:], cos_d[:])
            nc.sync.dma_start(sin_sb[:], sin_d[:])
            nc.sync.dma_start(mask_sb[:], mask_d[:])
            nc.sync.dma_start(ones_sb[:], ones_d[:])
            nc.sync.dma_start(eye_sb[:], eye_d[:])

            _XTB = {}

            def load_xt(c):
                """X^T chunk c -> SBUF [128, kt*TC], 4 parallel DMAs."""
                xtb = sb.tile([128, KT * TC], BF16, tag="xtb", bufs=1,
                              name=f"xtb{c}")
                for q in range(4):
                    k0, k1 = q * (KT // 4), (q + 1) * (KT // 4)
                    nc.sync.dma_start(
                        xtb.rearrange("p (kt t) -> p kt t", t=TC)[:, k0:k1],
                        xt_d.rearrange("p (kt t) -> p kt t", t=S)
                        [:, k0:k1, c * TC:(c + 1) * TC])
                _XTB[c] = xtb

            def get_panel(idx, eng=None):
                wb = sb.tile([128, PANW], BF16, tag="wb", bufs=5, name="wb")
                (eng or nc.sync).dma_start(
                    wb[:], w_d[:, idx * PANW:(idx + 1) * PANW])
                return wb

            def rope(dst, pq, t0):
                """dst (bf16 [128, TC]) = rope of pq (fp32 PSUM [128, TC])."""
                qf = sb.tile([128, TC], F32, tag="qf", bufs=2)
                nc.scalar.copy(qf[:], pq[:])
                qs = sb.tile([128, TC], F32, tag="qs", bufs=2)
                nc.sync.dma_start(qs[0:64, :], qf[64:128, :])
                nc.sync.dma_start(qs[64:128, :], qf[0:64, :])
                nc.vector.tensor_tensor(
                    qf[:], qf[:], cos_sb[:, t0:t0 + TC], mybir.AluOpType.mult)
                nc.vector.tensor_tensor(
                    qs[:], qs[:], sin_sb[:, t0:t0 + TC], mybir.AluOpType.mult)
                nc.vector.tensor_tensor(dst, qf[:], qs[:], mybir.AluOpType.add)

            def proj(c):
                t0 = c * TC
                xtb = _XTB[c]
                qtb = sb.tile([128, HL * TC], BF16, tag="qt", bufs=4, name="qtb")
                for h in range(HL):
                    wb = get_panel(h)
                    pq = ps.tile([128, TC], F32, tag="pj", bufs=2, name="pq")
                    for kt in range(KT):
                        nc.tensor.matmul(
                            pq[:], wb[:, kt * 128:(kt + 1) * 128],
                            xtb[:, kt * TC:(kt + 1) * TC],
                            start=(kt == 0), stop=(kt == KT - 1))
                    rope(qtb[:, h * TC:(h + 1) * TC], pq, t0)
                for kv in range(KVL):
                    wb = get_panel(8 + kv)
                    pk = ps.tile([128, TC], F32, tag="pj", bufs=2, name="pk")
                    for kt in range(KT):
                        nc.tensor.matmul(
                            pk[:], wb[:, kt * 128:(kt + 1) * 128],
                            xtb[:, kt * TC:(kt + 1) * TC],
                            start=(kt == 0), stop=(kt == KT - 1))
                    rope(ktb[:, kv * S + t0:kv * S + t0 + TC], pk, t0)
                for kv in range(KVL):
                    wb = get_panel(10 + kv)
                    pv = ps.tile([128, TC], F32, tag="pj", bufs=2, name="pv")
                    for kt in range(KT):
                        nc.tensor.matmul(
                            pv[:], wb[:, kt * 128:(kt + 1) * 128],
                            xtb[:, kt * TC:(kt + 1) * TC],
                            start=(kt == 0), stop=(kt == KT - 1))
                    vtb = sb.tile([128, TC], BF16, tag="vtb", bufs=2, name="vtb")
                    nc.scalar.copy(vtb[:], pv[:])
                    for tb in range(TC // 128):
                        tr = ps.tile([128, 128], BF16, tag="pdtr", bufs=2,
                                     name="tr")
                        nc.tensor.transpose(
                            tr[:], vtb[:, tb * 128:(tb + 1) * 128], eye_sb[:])
                        vt_idx = 4 * c + tb
                        nc.scalar.copy(
                            vb[:, vt_idx * (KVL * 128) + kv * 128:
                               vt_idx * (KVL * 128) + (kv + 1) * 128], tr[:])
                return qtb

            def attention(c, qtb):
                nkt = 4 * c + 4
                attnb = sb.tile([128, HL * TC], BF16, tag="attn", bufs=2,
                                name="attnb")
                ccos = []
                for h in range(HL):
                    kv = h // (HL // KVL)
                    qsl = qtb[:, h * TC:(h + 1) * TC]
                    pa = ps.tile([128, TC], F32, tag="pa", bufs=2, name="pa")
                    pd = ps.tile([128, TC], F32, tag="pdtr", bufs=2, name="pd")
                    pts = {}

                    def qoff(kt):
                        j = kt - 4 * c
                        return 128 * j if j >= 0 else 0

                    def qk_exp(kt):
                        o = qoff(kt)
                        sps = ps.tile([128, TC], F32, tag="s", bufs=2,
                                      name="sps")
                        nc.tensor.matmul(
                            sps[:, o:],
                            ktb[:, kv * S + kt * 128:kv * S + (kt + 1) * 128],
                            qsl[:, o:], start=True, stop=True)
                        pt = sb.tile([128, TC], BF16, tag="pt", bufs=3,
                                     name="pt")
                        nc.scalar.activation(
                            pt[:, o:], sps[:, o:],
                            mybir.ActivationFunctionType.Exp, scale=SCALE)
                        if kt - 4 * c >= 0:
                            nc.gpsimd.tensor_tensor(
                                pt[:, o:o + 128], pt[:, o:o + 128], mask_sb[:],
                                mybir.AluOpType.mult)
                        pts[kt] = pt

                    def pv_den(kt):
                        o = qoff(kt)
                        pt = pts.pop(kt)
                        nc.tensor.matmul(
                            pa[:, o:],
                            vb[:, kt * (KVL * 128) + kv * 128:
                               kt * (KVL * 128) + (kv + 1) * 128],
                            pt[:, o:], start=(kt == 0), stop=(kt == nkt - 1))
                        nc.tensor.matmul(
                            pd[:, o:], ones_sb[:], pt[:, o:],
                            start=(kt == 0), stop=(kt == nkt - 1))

                    qk_exp(0)
                    if nkt > 1:
                        qk_exp(1)
                    for kt in range(2, nkt):
                        pv_den(kt - 2)
                        qk_exp(kt)
                    if nkt > 1:
                        pv_den(nkt - 2)
                    pv_den(nkt - 1)

                    rc = sb.tile([128, TC], F32, tag="rc", bufs=2, name="rc")
                    nc.vector.reciprocal_approx_fast(rc[:], pd[:])
                    nc.vector.tensor_tensor(
                        attnb[:, h * TC:(h + 1) * TC], pa[:], rc[:],
                        mybir.AluOpType.mult)
                    if h == 3 or h == 7:
                        ccos.append(gather_piece(attnb, h - 3))
                return ccos

            def gather_piece(attnb, h0):
                """AllGather heads [h0, h0+4) of attnb -> cco [4*4*128, TC]."""
                cci = dr.tile([4 * 128, TC], BF16, tag="cci", bufs=8,
                              name="cci")
                cco = dr.tile([TP * 4 * 128, TC], BF16, tag="cco", bufs=8,
                              name="cco")
                nc.sync.dma_start(
                    cci.rearrange("(h p) t -> p h t", p=128),
                    attnb.rearrange("p (h t) -> p h t", h=HL)[:, h0:h0 + 4])
                nc.gpsimd.collective_compute(
                    "AllGather", mybir.AluOpType.bypass,
                    replica_groups=GROUPS, ins=[cci[:]], outs=[cco[:]])
                return cco

            def load_af(cco, floor_ms):
                """cco piece -> SBUF [128, 16*TC] (g = r*4 + local head).

                floor_ms is a scheduler hint: the tile list-scheduler's sim
                has no model of AllGather latency, so without a floor it
                schedules af-consuming matmuls right after the collective
                trigger and the PE stalls on the real ~30us collective."""
                af = sb.tile([128, 16 * TC], BF16, tag="af", bufs=2, name="af")
                with tc.tile_wait_until(floor_ms):
                    nc.sync.dma_start(
                        af.rearrange("p (g t) -> p g t", g=16),
                        cco.rearrange("(g p) t -> p g t", p=128))
                return af

            def af_slice(afs, dt):
                """af slice for global d-tile dt (wo row-block r*8+hh)."""
                r, hh = dt // 8, dt % 8
                af = afs[hh // 4]
                slot = r * 4 + (hh % 4)
                return af[:, slot * TC:(slot + 1) * TC]

            def outproj(c, ccos, floors):
                t0 = c * TC
                afs = [load_af(ccos[0], floors[0]), load_af(ccos[1], floors[1])]
                for o in range(OC // 128):
                    wb = get_panel(12 + o)
                    po = ps.tile([128, TC], F32, tag="pj", bufs=2, name="po")
                    for i, dt in enumerate(range(KT)):
                        nc.tensor.matmul(
                            po[:], wb[:, dt * 128:(dt + 1) * 128],
                            af_slice(afs, dt),
                            start=(i == 0), stop=(i == KT - 1))
                    ot = sb.tile([128, TC], BF16, tag="ot", bufs=2, name="ot")
                    nc.vector.tensor_copy(ot[:], po[:])
                    nc.sync.dma_start(
                        out_d[o * 128:(o + 1) * 128, t0:t0 + TC], ot[:])

            # ---- schedule ----
            # All projections first (weights stream, X^T chunks prefetch),
            # then attentions ordered so the cheapest (chunk 0) runs last
            # and each AllGather is covered by the next attention or a
            # pending out-projection; the tail outproj runs in two
            # per-piece passes so the last collective overlaps matmuls.
            load_xt(0)
            qtbs = {}
            for c in range(NCHUNK):
                qtbs[c] = proj(c)
                if c + 1 < NCHUNK:
                    load_xt(c + 1)
            gath = {}
            gath[1] = attention(1, qtbs[1])
            gath[2] = attention(2, qtbs[2])
            gath[3] = attention(3, qtbs[3])
            outproj(1, gath[1], (0.55, 0.57))
            gath[0] = attention(0, qtbs[0])
            outproj(2, gath[2], (0.72, 0.74))
            outproj(3, gath[3], (0.80, 0.82))
            outproj(0, gath[0], (0.88, 0.90))

    nc.compile()
    return nc


def _get_nc():
    if "nc" not in _BUILT:
        _BUILT["nc"] = _build()
    return _BUILT["nc"]


def _to_bf16(a):
    return np.ascontiguousarray(a.astype(ml_dtypes.bfloat16))


def _panelize(w):
    """[H, 128] fp32 -> [128, KT*128] bf16 in [p, kt, c] layout."""
    return _to_bf16(
        w.reshape(KT, 128, 128).transpose(1, 0, 2).reshape(128, PANW))


def kernel(hidden_states, cos, sin, wq, wk, wv, wo):
    global LAST_RESULT
    nc = _get_nc()

    hidden_states = np.asarray(hidden_states, dtype=np.float32)
    cos = np.asarray(cos, dtype=np.float32)
    sin = np.asarray(sin, dtype=np.float32)
    wq = np.asarray(wq, dtype=np.float32)
    wk = np.asarray(wk, dtype=np.float32)
    wv = np.asarray(wv, dtype=np.float32)
    wo = np.asarray(wo, dtype=np.float32)

    # host-side shard prep: X^T as [128, kt*S] bf16
    xts = []
    for b in range(B):
        xt = hidden_states[b].T.reshape(KT, 128, S).transpose(1, 0, 2)
        xts.append(_to_bf16(xt.reshape(128, KT * S)))
    cts = [np.ascontiguousarray(cos[b].T) for b in range(B)]
    sin_eff = []
    for b in range(B):
        st = np.ascontiguousarray(sin[b].T)
        se = st.copy()
        se[0:64, :] *= -1.0
        sin_eff.append(se)

    maskb = np.triu(np.ones((128, 128))).astype(ml_dtypes.bfloat16)
    onesb = np.ones((128, 128), dtype=ml_dtypes.bfloat16)
    eyeb = np.eye(128).astype(ml_dtypes.bfloat16)

    # weight panels per tp rank: 8 wq, 2 wk, 2 wv, 8 wo
    w_alls = []
    for r in range(TP):
        panels = []
        for h in range(HL):
            panels.append(_panelize(wq[:, r * QCOLS + h * 128:
                                       r * QCOLS + (h + 1) * 128]))
        for kv in range(KVL):
            panels.append(_panelize(wk[:, r * (KVL * 128) + kv * 128:
                                       r * (KVL * 128) + (kv + 1) * 128]))
        for kv in range(KVL):
            panels.append(_panelize(wv[:, r * (KVL * 128) + kv * 128:
                                       r * (KVL * 128) + (kv + 1) * 128]))
        for o in range(OC // 128):
            panels.append(_panelize(wo[:, r * OC + o * 128:
                                       r * OC + (o + 1) * 128]))
        w_alls.append(np.concatenate(panels, axis=1))

    in_maps = []
    for core in range(N_CORES):
        b, r = core // TP, core % TP
        in_maps.append({
            "xt": xts[b],
            "w_all": w_alls[r],
            "cos_t": cts[b],
            "sin_t": sin_eff[b],
            "maskb": maskb,
            "onesb": onesb,
            "eyeb": eyeb,
        })

    res = run_bass_kernel_spmd(nc, in_maps, core_ids=list(range(N_CORES)))
    LAST_RESULT = res

    out = np.empty((B, S, H), dtype=np.float32)
    for core in range(N_CORES):
        b, r = core // TP, core % TP
        out[b, :, r * OC:(r + 1) * OC] = \
            res.results[core]["out_t"].astype(np.float32).T
    return out


# revision 26
# speedup vs baseline: 1.0362x; 1.0362x over previous
"""Llama GQA attention (B=2, S=2048, H=4096, 32 q heads / 8 kv heads, HD=128)
on 8 Trainium2 NeuronCores.

Sharding: DP=2 over batch x TP=4 over heads.
  core c: batch b = c // 4, tp rank r = c % 4
  - owns q heads [8r, 8r+8), kv heads [2r, 2r+2)
  - AllGather (bf16) of attention outputs within each 4-core batch group,
    split in two 4-head pieces triggered mid-attention
  - output projection over the full 4096 attn features for output columns
    [1024r, 1024(r+1)) -> disjoint outputs, host concatenates.

All inputs are pre-cast/pre-transposed to the on-chip layout on the HOST
(bf16 weights/X^T), so the device does zero dtype conversion and reads
every operand exactly once per use:
  xt:  [128, kt*S + t]   X^T with hidden dim split into 32 k-tiles
  w:   [128, panel*4096] 20 weight panels (8 wq, 2 wk, 2 wv^T, 8 wo),
                          each [128 k-part, kt*128 + c]
On-chip layout is fully "transposed" ([feature, token]); V^T is computed
with free=512 matmuls like K^T and flipped to V via PE transposes.
Causal masking: per q-chunk c only k-tiles 0..4c+3 are touched; the 4
diagonal tiles compute only the surviving q-range (free = 512-128j) plus
one static [128,128] triangular mask on the leading square (GpSimd).
Softmax skips max-subtraction (scores are O(8), exp fits bf16), the
denominator accumulates on the PE via a ones-matmul, and the reciprocal
uses the fast DVE approximation (18 bits).

Schedule: all four projections first (weight panels stream from DRAM
with a 3-deep prefetch, X^T chunks load between them), then attentions
ordered so the cheapest (chunk 0) runs last, with out-projections
interleaved late: attn1, attn2, attn3, op1, attn0, op2, op3, op0.
Each AllGather piece (triggered after heads 3 and 7 of its attention)
gets 130-170us of covering work, making the kernel robust to
collective-latency variance. The af loads carry tile_wait_until floors
so the list scheduler (whose sim has no collective-latency model) does
not hoist af-consuming matmuls ahead of available attention work.
"""

import sys

for _p in ("/opt/trn_rl_repo",):
    if _p not in sys.path:
        sys.path.append(_p)

import numpy as np
import ml_dtypes

import concourse.bacc as bacc
import concourse.mybir as mybir
import concourse.tile as tile
from concourse.bass_utils import run_bass_kernel_spmd

F32 = mybir.dt.float32
BF16 = mybir.dt.bfloat16

B, S, H = 2, 2048, 4096
NH, NKV, HD = 32, 8, 128
N_CORES = 8
TP = 4
GROUPS = [[0, 1, 2, 3], [4, 5, 6, 7]]

HL = NH // TP          # 8 local q heads
KVL = NKV // TP        # 2 local kv heads
QCOLS = HL * HD        # 1024 local q cols
OC = H // TP           # 1024 local out cols

TC = 512               # token chunk (= one attention q-block)
NCHUNK = S // TC       # 4
KT = H // 128          # 32 contraction tiles
SCALE = float(HD ** -0.5)
NPANEL = 20            # 8 wq + 2 wk + 2 wv + 8 wo
PANW = KT * 128        # 4096 free cols per weight panel

LAST_RESULT = None
_BUILT = {}


def _build():
    nc = bacc.Bacc("TRN2", debug=False, num_devices=N_CORES)

    xt_d = nc.dram_tensor("xt", [128, KT * S], BF16, kind="ExternalInput").ap()
    w_d = nc.dram_tensor("w_all", [128, NPANEL * PANW], BF16,
                         kind="ExternalInput").ap()
    cos_d = nc.dram_tensor("cos_t", [HD, S], F32, kind="ExternalInput").ap()
    sin_d = nc.dram_tensor("sin_t", [HD, S], F32, kind="ExternalInput").ap()
    mask_d = nc.dram_tensor("maskb", [128, 128], BF16, kind="ExternalInput").ap()
    ones_d = nc.dram_tensor("onesb", [128, 128], BF16, kind="ExternalInput").ap()
    eye_d = nc.dram_tensor("eyeb", [128, 128], BF16, kind="ExternalInput").ap()
    out_d = nc.dram_tensor("out_t", [OC, S], F32, kind="ExternalOutput").ap()

    with tile.TileContext(nc) as tc:
        with tc.tile_pool(name="sb", bufs=1) as sb, \
             tc.tile_pool(name="ps", bufs=1, space="PSUM") as ps, \
             tc.tile_pool(name="dr", bufs=1, space="DRAM") as dr:

            # ---- persistent tiles ----
            cos_sb = sb.tile([HD, S], F32)
            sin_sb = sb.tile([HD, S], F32)
            mask_sb = sb.tile([128, 128], BF16)
            ones_sb = sb.tile([128, 128], BF16)
            eye_sb = sb.tile([128, 128], BF16)
            ktb = sb.tile([128, KVL * S], BF16)            # roped K^T [d, kv*S+t]
            vb = sb.tile([128, (S // 128) * KVL * 128], BF16)  # V [t, tt*256+d]
            nc.sync.dma_start(cos_sb[:], cos_d[:])
            nc.sync.dma_start(sin_sb[:], sin_d[:])
            nc.sync.dma_start(mask_sb[:], mask_d[:])
            nc.sync.dma_start(ones_sb[:], ones_d[:])
            nc.sync.dma_start(eye_sb[:], eye_d[:])

            _XTB = {}

            def load_xt(c):
                """X^T chunk c -> SBUF [128, kt*TC], 4 parallel DMAs."""
                xtb = sb.tile([128, KT * TC], BF16, tag="xtb", bufs=1,
                              name=f"xtb{c}")
                for q in range(4):
                    k0, k1 = q * (KT // 4), (q + 1) * (KT // 4)
                    nc.sync.dma_start(
                        xtb.rearrange("p (kt t) -> p kt t", t=TC)[:, k0:k1],
                        xt_d.rearrange("p (kt t) -> p kt t", t=S)
                        [:, k0:k1, c * TC:(c + 1) * TC])
                _XTB[c] = xtb

            def get_panel(idx, eng=None):
                wb = sb.tile([128, PANW], BF16, tag="wb", bufs=4, name="wb")
                (eng or nc.sync).dma_start(
                    wb[:], w_d[:, idx * PANW:(idx + 1) * PANW])
                return wb

            def rope(dst, pq, t0):
                """dst (bf16 [128, TC]) = rope of pq (fp32 PSUM [128, TC])."""
                qf = sb.tile([128, TC], F32, tag="qf", bufs=2)
                nc.scalar.copy(qf[:], pq[:])
                qs = sb.tile([128, TC], F32, tag="qs", bufs=2)
                nc.sync.dma_start(qs[0:64, :], qf[64:128, :])
                nc.sync.dma_start(qs[64:128, :], qf[0:64, :])
                nc.vector.tensor_tensor(
                    qf[:], qf[:], cos_sb[:, t0:t0 + TC], mybir.AluOpType.mult)
                nc.vector.tensor_tensor(
                    qs[:], qs[:], sin_sb[:, t0:t0 + TC], mybir.AluOpType.mult)
                nc.vector.tensor_tensor(dst, qf[:], qs[:], mybir.AluOpType.add)

            def proj(c):
                t0 = c * TC
                xtb = _XTB[c]
                qtb = sb.tile([128, HL * TC], BF16, tag="qt", bufs=4, name="qtb")
                for h in range(HL):
                    wb = get_panel(h)
                    pq = ps.tile([128, TC], F32, tag="pj", bufs=2, name="pq")
                    for kt in range(KT):
                        nc.tensor.matmul(
                            pq[:], wb[:, kt * 128:(kt + 1) * 128],
                            xtb[:, kt * TC:(kt + 1) * TC],
                            start=(kt == 0), stop=(kt == KT - 1))
                    rope(qtb[:, h * TC:(h + 1) * TC], pq, t0)
                for kv in range(KVL):
                    wb = get_panel(8 + kv)
                    pk = ps.tile([128, TC], F32, tag="pj", bufs=2, name="pk")
                    for kt in range(KT):
                        nc.tensor.matmul(
                            pk[:], wb[:, kt * 128:(kt + 1) * 128],
                            xtb[:, kt * TC:(kt + 1) * TC],
                            start=(kt == 0), stop=(kt == KT - 1))
                    rope(ktb[:, kv * S + t0:kv * S + t0 + TC], pk, t0)
                for kv in range(KVL):
                    wb = get_panel(10 + kv)
                    pv = ps.tile([128, TC], F32, tag="pj", bufs=2, name="pv")
                    for kt in range(KT):
                        nc.tensor.matmul(
                            pv[:], wb[:, kt * 128:(kt + 1) * 128],
                            xtb[:, kt * TC:(kt + 1) * TC],
                            start=(kt == 0), stop=(kt == KT - 1))
                    vtb = sb.tile([128, TC], BF16, tag="vtb", bufs=2, name="vtb")
                    nc.scalar.copy(vtb[:], pv[:])
                    for tb in range(TC // 128):
                        tr = ps.tile([128, 128], BF16, tag="pdtr", bufs=2,
                                     name="tr")
                        nc.tensor.transpose(
                            tr[:], vtb[:, tb * 128:(tb + 1) * 128], eye_sb[:])
                        vt_idx = 4 * c + tb
                        nc.scalar.copy(
                            vb[:, vt_idx * (KVL * 128) + kv * 128:
                               vt_idx * (KVL * 128) + (kv + 1) * 128], tr[:])
                return qtb

            def attention(c, qtb):
                nkt = 4 * c + 4
                attnb = sb.tile([128, HL * TC], BF16, tag="attn", bufs=2,
                                name="attnb")
                ccos = []
                for h in range(HL):
                    kv = h // (HL // KVL)
                    qsl = qtb[:, h * TC:(h + 1) * TC]
                    pa = ps.tile([128, TC], F32, tag="pa", bufs=2, name="pa")
                    pd = ps.tile([128, TC], F32, tag="pdtr", bufs=2, name="pd")
                    pts = {}

                    def qoff(kt):
                        j = kt - 4 * c
                        return 128 * j if j >= 0 else 0

                    def qk_exp(kt):
                        o = qoff(kt)
                        sps = ps.tile([128, TC], F32, tag="s", bufs=2,
                                      name="sps")
                        nc.tensor.matmul(
                            sps[:, o:],
                            ktb[:, kv * S + kt * 128:kv * S + (kt + 1) * 128],
                            qsl[:, o:], start=True, stop=True)
                        pt = sb.tile([128, TC], BF16, tag="pt", bufs=3,
                                     name="pt")
                        nc.scalar.activation(
                            pt[:, o:], sps[:, o:],
                            mybir.ActivationFunctionType.Exp, scale=SCALE)
                        if kt - 4 * c >= 0:
                            nc.gpsimd.tensor_tensor(
                                pt[:, o:o + 128], pt[:, o:o + 128], mask_sb[:],
                                mybir.AluOpType.mult)
                        pts[kt] = pt

                    def pv_den(kt):
                        o = qoff(kt)
                        pt = pts.pop(kt)
                        nc.tensor.matmul(
                            pa[:, o:],
                            vb[:, kt * (KVL * 128) + kv * 128:
                               kt * (KVL * 128) + (kv + 1) * 128],
                            pt[:, o:], start=(kt == 0), stop=(kt == nkt - 1))
                        nc.tensor.matmul(
                            pd[:, o:], ones_sb[:], pt[:, o:],
                            start=(kt == 0), stop=(kt == nkt - 1))

                    qk_exp(0)
                    if nkt > 1:
                        qk_exp(1)
                    for kt in range(2, nkt):
                        pv_den(kt - 2)
                        qk_exp(kt)
                    if nkt > 1:
                        pv_den(nkt - 2)
                    pv_den(nkt - 1)

                    rc = sb.tile([128, TC], F32, tag="rc", bufs=2, name="rc")
                    nc.vector.reciprocal_approx_fast(rc[:], pd[:])
                    nc.vector.tensor_tensor(
                        attnb[:, h * TC:(h + 1) * TC], pa[:], rc[:],
                        mybir.AluOpType.mult)
                    if h == 3 or h == 7:
                        ccos.append(gather_piece(attnb, h - 3))
                return ccos

            def gather_piece(attnb, h0):
                """AllGather heads [h0, h0+4) of attnb -> cco [4*4*128, TC]."""
                cci = dr.tile([4 * 128, TC], BF16, tag="cci", bufs=8,
                              name="cci")
                cco = dr.tile([TP * 4 * 128, TC], BF16, tag="cco", bufs=8,
                              name="cco")
                nc.sync.dma_start(
                    cci.rearrange("(h p) t -> p h t", p=128),
                    attnb.rearrange("p (h t) -> p h t", h=HL)[:, h0:h0 + 4])
                nc.gpsimd.collective_compute(
                    "AllGather", mybir.AluOpType.bypass,
                    replica_groups=GROUPS, ins=[cci[:]], outs=[cco[:]])
                return cco

            def load_af(cco, floor_ms):
                """cco piece -> SBUF [128, 16*TC] (g = r*4 + local head).

                floor_ms is a scheduler hint: the tile list-scheduler's sim
                has no model of AllGather latency, so without a floor it
                schedules af-consuming matmuls right after the collective
                trigger and the PE stalls on the real ~30us collective."""
                af = sb.tile([128, 16 * TC], BF16, tag="af", bufs=2, name="af")
                with tc.tile_wait_until(floor_ms):
                    nc.sync.dma_start(
                        af.rearrange("p (g t) -> p g t", g=16),
                        cco.rearrange("(g p) t -> p g t", p=128))
                return af

            def af_slice(afs, dt):
                """af slice for global d-tile dt (wo row-block r*8+hh)."""
                r, hh = dt // 8, dt % 8
                af = afs[hh // 4]
                slot = r * 4 + (hh % 4)
                return af[:, slot * TC:(slot + 1) * TC]

            def outproj(c, ccos, floors):
                t0 = c * TC
                afs = [load_af(ccos[0], floors[0]), load_af(ccos[1], floors[1])]
                for o in range(OC // 128):
                    wb = get_panel(12 + o)
                    po = ps.tile([128, TC], F32, tag="pj", bufs=2, name="po")
                    for i, dt in enumerate(range(KT)):
                        nc.tensor.matmul(
                            po[:], wb[:, dt * 128:(dt + 1) * 128],
                            af_slice(afs, dt),
                            start=(i == 0), stop=(i == KT - 1))
                    ot = sb.tile([128, TC], F32, tag="ot", bufs=2, name="ot")
                    nc.vector.tensor_copy(ot[:], po[:])
                    nc.sync.dma_start(
                        out_d[o * 128:(o + 1) * 128, t0:t0 + TC], ot[:])

            # ---- schedule ----
            # All projections first (weights stream, X^T chunks prefetch),
            # then attentions ordered so the cheapest (chunk 0) runs last
            # and each AllGather is covered by the next attention or a
            # pending out-projection; the tail outproj runs in two
            # per-piece passes so the last collective overlaps matmuls.
            load_xt(0)
            qtbs = {}
            for c in range(NCHUNK):
                qtbs[c] = proj(c)
                if c + 1 < NCHUNK:
                    load_xt(c + 1)
            gath = {}
            gath[1] = attention(1, qtbs[1])
            gath[2] = attention(2, qtbs[2])
            gath[3] = attention(3, qtbs[3])
            outproj(1, gath[1], (0.50, 0.53))
            gath[0] = attention(0, qtbs[0])
            outproj(2, gath[2], (0.68, 0.70))
            outproj(3, gath[3], (0.76, 0.78))
            outproj(0, gath[0], (0.84, 0.86))

    nc.compile()
    return nc


def _get_nc():
    if "nc" not in _BUILT:
        _BUILT["nc"] = _build()
    return _BUILT["nc"]


def _to_bf16(a):
    return np.ascontiguousarray(a.astype(ml_dtypes.bfloat16))


def _panelize(w):
    """[H, 128] fp32 -> [128, KT*128] bf16 in [p, kt, c] layout."""
    return _to_bf16(
        w.reshape(KT, 128, 128).transpose(1, 0, 2).reshape(128, PANW))


def kernel(hidden_states, cos, sin, wq, wk, wv, wo):
    global LAST_RESULT
    nc = _get_nc()

    hidden_states = np.asarray(hidden_states, dtype=np.float32)
    cos = np.asarray(cos, dtype=np.float32)
    sin = np.asarray(sin, dtype=np.float32)
    wq = np.asarray(wq, dtype=np.float32)
    wk = np.asarray(wk, dtype=np.float32)
    wv = np.asarray(wv, dtype=np.float32)
    wo = np.asarray(wo, dtype=np.float32)

    # host-side shard prep: X^T as [128, kt*S] bf16
    xts = []
    for b in range(B):
        xt = hidden_states[b].T.reshape(KT, 128, S).transpose(1, 0, 2)
        xts.append(_to_bf16(xt.reshape(128, KT * S)))
    cts = [np.ascontiguousarray(cos[b].T) for b in range(B)]
    sin_eff = []
    for b in range(B):
        st = np.ascontiguousarray(sin[b].T)
        se = st.copy()
        se[0:64, :] *= -1.0
        sin_eff.append(se)

    maskb = np.triu(np.ones((128, 128))).astype(ml_dtypes.bfloat16)
    onesb = np.ones((128, 128), dtype=ml_dtypes.bfloat16)
    eyeb = np.eye(128).astype(ml_dtypes.bfloat16)

    # weight panels per tp rank: 8 wq, 2 wk, 2 wv, 8 wo
    w_alls = []
    for r in range(TP):
        panels = []
        for h in range(HL):
            panels.append(_panelize(wq[:, r * QCOLS + h * 128:
                                       r * QCOLS + (h + 1) * 128]))
        for kv in range(KVL):
            panels.append(_panelize(wk[:, r * (KVL * 128) + kv * 128:
                                       r * (KVL * 128) + (kv + 1) * 128]))
        for kv in range(KVL):
            panels.append(_panelize(wv[:, r * (KVL * 128) + kv * 128:
                                       r * (KVL * 128) + (kv + 1) * 128]))
        for o in range(OC // 128):
            panels.append(_panelize(wo[:, r * OC + o * 128:
                                       r * OC + (o + 1) * 128]))
        w_alls.append(np.concatenate(panels, axis=1))

    in_maps = []
    for core in range(N_CORES):
        b, r = core // TP, core % TP
        in_maps.append({
            "xt": xts[b],
            "w_all": w_alls[r],
            "cos_t": cts[b],
            "sin_t": sin_eff[b],
            "maskb": maskb,
            "onesb": onesb,
            "eyeb": eyeb,
        })

    res = run_bass_kernel_spmd(nc, in_maps, core_ids=list(range(N_CORES)))
    LAST_RESULT = res

    out = np.empty((B, S, H), dtype=np.float32)
    for core in range(N_CORES):
        b, r = core // TP, core % TP
        out[b, :, r * OC:(r + 1) * OC] = res.results[core]["out_t"].T
    return out
